# revision 51
# baseline (speedup 1.0000x reference)
"""MultiHeadedAttention Trainium2 Bass kernel (v3, optimized).

Full inputs in, full output out. Sharding: 8 cores = 4 batches x 2 head-pairs
(data-parallel over batch, tensor-parallel over the 4 heads).

Per core (batch b, heads h0/h1), all matmuls bf16 except the fp8 x-accum:
  - weights DMA'd FIRST (tiny; before the 3MB input stream hogs the DMA
    engines), inputs window-contiguous (2KB/partition descriptors) across
    the sync+gpsimd queues; K/V/Q projections stream in arrival order,
    interleaved into the first attention chunk via deferred emission.
  - Q/K proj -> q_sb/k_sb [128 (h,d), 2048] bf16; bias folded into the
    mandatory PSUM->SBUF copy (ACT Identity bias AP / DVE tensor_scalar).
    V bias folded into the host-side output bias (out += wm @ bv).
  - V proj -> vt [128 m, 8 j, 2 h, 2 i, 80] fp8e4: DoubleRow pair layout,
    slot j = m-blocks (2j, 2j+1), i = mb%2; col 64 = ones (softmax sums),
    cols 65:79 zero pad (16B-aligned DR weights AP, M=80).
  - scores: row-tiled matmul pairs via tile_position (h0 rows 0:63, h1
    rows 64:127) -> two adjacent PSUM banks [128, 1024]; a duplicate
    h0-scores matmul per step keeps the PE HAM clock gate at 2.4GHz.
  - exp: ONE instruction per (c4, mb) covering both heads' banks, split
    5/8 ACT : 3/8 DVE per pair-slot. ACT: exp(0.125*s + ln2) -> fp8e4.
    DVE: Schraudolph bit-trick int8(s/ln2 + 63.8) bitcast as fp8e4 (same
    2x scale; scale cancels in softmax). Per-slot pt tiles [128,2,2,512]
    keep the exp->DR pipeline free of tile-granular WAR serialization.
  - x-accum: fp8 DoubleRow matmul per (h, j): K=256 virtual, M=80, N=512;
    PSUM accumulates over j; px row 64 = softmax sums.
  - normalize: copy px->SBUF; sums broadcast via DRAM-bounce DMA (chunks
    0-2, off the PE, pipelined into the next chunk) or a fp16 ones matmul
    (last chunk, low latency); reciprocal_approx_fast (DVE); multiply on
    gpsimd -> xcat [128 (h,d), 512] bf16.
  - out-proj: lhsT=wmcat [128 (h,d), 128 oc] (heads fused, K=128) per
    512-wide n chunk; out fp32 DMA'd per chunk on gpsimd/sync queues;
    host sums the two per-batch partials + effective bias.
Tail work of chunk c4 is emitted at deferred positions inside chunk c4+1's
mb loop (in-order engines: head-of-line blocking otherwise).
"""

import sys

if "/opt/trn_rl_repo" not in sys.path:
    sys.path.insert(0, "/opt/trn_rl_repo")

import numpy as np
import ml_dtypes

BF = ml_dtypes.bfloat16
F8 = ml_dtypes.float8_e4m3fn

B, D, N, H = 4, 256, 2048, 4
DIM = D // H  # 64
NW = 4   # 512-wide input windows
MB = 16  # 128-wide m blocks
LN2 = 0.6931471805599453
SCHRAUD_A = 1.0 / LN2          # bits = A*s + B  (score -> fp8e4 bit pattern)
SCHRAUD_B = 63.8               # 64 = x2 scale (matches ACT's +ln2 bias)

_CACHE = {}

import os
CFG_DR = os.environ.get("K_DR", "1") == "1"          # DoubleRow x-accum
CFG_SCHRAUD = os.environ.get("K_SCHRAUD", "1") == "1"  # DVE bit-trick exp
CFG_ACT_FP8 = os.environ.get("K_ACT_FP8", "1") == "1"  # ACT exp -> fp8 out
CFG_TILEPOS = os.environ.get("K_TILEPOS", "1") == "1"  # row-tiled scores
CFG_FILLER = os.environ.get("K_FILLER", "1") == "1"   # HAM keep-warm fillers
CFG_WARMUP = int(os.environ.get("K_WARMUP", "4"))
CFG_KEEPWARM = int(os.environ.get("K_KEEPWARM", "1"))


def _emit(ctx, tc, io):
    import concourse.bass as bass
    import concourse.mybir as mybir

    nc = tc.nc
    f32 = mybir.dt.float32
    bf16 = mybir.dt.bfloat16
    fp8 = mybir.dt.float8e4
    i8 = mybir.dt.int8
    EXP = mybir.ActivationFunctionType.Exp
    IDENT = mybir.ActivationFunctionType.Identity
    MUL = mybir.AluOpType.mult
    ADD = mybir.AluOpType.add
    DR = mybir.MatmulPerfMode.DoubleRow

    const = ctx.enter_context(tc.tile_pool(name="const", bufs=1))
    xin = ctx.enter_context(tc.tile_pool(name="xin", bufs=12))
    big = ctx.enter_context(tc.tile_pool(name="big", bufs=1))
    ptp = ctx.enter_context(tc.tile_pool(name="ptp", bufs=6))
    work = ctx.enter_context(tc.tile_pool(name="work", bufs=4))
    xcp = ctx.enter_context(tc.tile_pool(name="xcp", bufs=2))
    outp = ctx.enter_context(tc.tile_pool(name="outp", bufs=4))
    psSC = ctx.enter_context(tc.tile_pool(name="psSC", bufs=2, space="PSUM"))
    psPX = ctx.enter_context(tc.tile_pool(name="psPX", bufs=2, space="PSUM"))
    psPR = ctx.enter_context(tc.tile_pool(name="psPR", bufs=2, space="PSUM"))
    dpool = ctx.enter_context(tc.tile_pool(name="dpool", bufs=4, space="DRAM"))

    # ---- weights FIRST (before the big input streams hog the DMA
    # engines): ~200KB total, lands in ~1us ----
    wqt_sb = const.tile([128, 2, 128], bf16, tag="wqt")
    nc.sync.dma_start(wqt_sb, io["wqt"])
    wkt_sb = const.tile([128, 2, 128], bf16, tag="wkt")
    nc.scalar.dma_start(wkt_sb, io["wkt"])
    bqk_sb = const.tile([128, 2], f32, tag="bqk")
    nc.scalar.dma_start(bqk_sb, io["bqk"])
    bq_sb = bqk_sb[:, 0:1]
    bk_sb = bqk_sb[:, 1:2]
    wvt_sb = const.tile([128, 2, 128], bf16, tag="wvt")
    nc.scalar.dma_start(wvt_sb, io["wvt"])
    wmcat_sb = const.tile([128, 2, 256], bf16, tag="wmcat")
    nc.scalar.dma_start(wmcat_sb, io["wmcat"])
    ones_h = const.tile([128, 64], mybir.dt.float16, tag="ones_h")
    nc.gpsimd.memset(ones_h, 1.0)
    ln2b = const.tile([128, 1], f32, tag="ln2b")
    nc.gpsimd.memset(ln2b, LN2)

    # ---- PE warmup: garbage matmuls release the HAM clock gate while the
    # input DMAs stream. Values are never read. ----
    wu = const.tile([128, 512], bf16, tag="wu")
    nc.vector.memset(wu[0:1, 0:1], 0.0)  # allocate; rest is garbage, never read
    wu_ps = psPR.tile([128, 512], f32, tag="pr", name="wu_ps")
    for _ in range(CFG_WARMUP):
        nc.tensor.matmul(wu_ps, lhsT=wu[:, 0:128], rhs=wu,
                         start=True, stop=True)

    # ---- input DMAs: window-contiguous host layout, 2KB/partition per
    # window. K first (scores m loop), Q w0 (first n chunk), V next. ----
    xq_t, xk_t, xv_t = [None] * NW, [None] * NW, [None] * NW
    order = [("xk", 0, nc.sync), ("xq", 0, nc.gpsimd), ("xv", 0, nc.sync),
             ("xk", 1, nc.gpsimd), ("xv", 1, nc.sync), ("xk", 2, nc.gpsimd),
             ("xq", 1, nc.sync), ("xv", 2, nc.gpsimd), ("xk", 3, nc.sync),
             ("xv", 3, nc.gpsimd), ("xq", 2, nc.sync), ("xq", 3, nc.gpsimd)]
    tiles = {"xq": xq_t, "xk": xk_t, "xv": xv_t}
    for name, w, eng in order:
        t = xin.tile([128, 2, 512], bf16, tag=name, name=f"{name}{w}")
        eng.dma_start(t, io[name][:, w, :, :])
        tiles[name][w] = t

    # ---- vt tile: cols 0:64 = v, col 64 = ones (softmax sums -> px row
    # 64), cols 65:79 zero pad (16-aligned DoubleRow weights AP, M=80;
    # px rows 65:79 dead). DR pair slot j covers m-blocks (2j, 2j+1),
    # i = mb % 2. ----
    vt = big.tile([128, 8, 2, 2, 80], fp8, tag="vt")
    for h in range(2):
        for i in range(2):
            nc.gpsimd.memset(vt[:, :, h, i, 65:80], 0.0)
            nc.gpsimd.memset(vt[:, :, h, i, 64:65], 1.0)

    q_sb = big.tile([128, 2048], bf16, tag="q")
    k_sb = big.tile([128, 2048], bf16, tag="k")

    # ---- projections (PE emission order = DMA arrival order) ----
    def keepwarm(ps, n=CFG_KEEPWARM):
        # dep-free garbage MMs into a dead region of a psum tile about to be
        # overwritten; keeps the PE HAM activity window busy during DMA waits
        for _ in range(n):
            nc.tensor.matmul(ps[0:32, :], lhsT=wu[:, 0:32], rhs=wu,
                             start=True, stop=True, skip_group_check=True)

    def qk_proj(xt, wt, bias, dst, w, use_act):
        ps = psPR.tile([128, 512], f32, tag="pr", name=f"ps_{dst.name}{w}")
        keepwarm(ps)
        nc.tensor.matmul(ps, lhsT=wt[:, 0, :], rhs=xt[w][:, 0, :],
                         start=True, stop=False)
        nc.tensor.matmul(ps, lhsT=wt[:, 1, :], rhs=xt[w][:, 1, :],
                         start=False, stop=True)
        ws = slice(w * 512, (w + 1) * 512)
        if use_act:
            nc.scalar.activation(dst[:, ws], ps, IDENT, bias=bias, scale=1.0)
        else:
            nc.vector.tensor_scalar_add(dst[:, ws], ps, bias)

    def v_proj(w, use_act):
        # 4 m-blocks (mb = 4w+r, pair slots j = 2w, 2w+1) -> one [128, 512]
        # psum -> two fp8 copies (one per i parity) into vt.
        ps = psPR.tile([128, 512], f32, tag="pr", name=f"ps_v{w}")
        keepwarm(ps)
        for r in range(4):
            ms = slice(r * 128, (r + 1) * 128)
            pvt = ps[:, r * 128:(r + 1) * 128]
            nc.tensor.matmul(pvt, lhsT=xv_t[w][:, 0, ms], rhs=wvt_sb[:, 0, :],
                             start=True, stop=False)
            nc.tensor.matmul(pvt, lhsT=xv_t[w][:, 1, ms], rhs=wvt_sb[:, 1, :],
                             start=False, stop=True)
        src = ps[:, :].rearrange("m (r h d) -> m r h d", r=4, h=2)
        for i in range(2):
            dst = vt[:, 2 * w:2 * w + 2, :, i, 0:64]
            if use_act:
                nc.scalar.copy(dst, src[:, i::2, :, :])
            else:
                nc.vector.tensor_copy(dst, src[:, i::2, :, :])

    # ---- attention: c4-outer (4 x 512-wide n chunks), mb inner.
    # K/V projections for windows 1..3 are emitted inside the c4=0 loop
    # (scores for m-window w only need them by mb=4w); Q windows 1..3 are
    # emitted at the ends of chunks 0..2. Tail work of chunk c4 is emitted
    # interleaved into chunk c4+1's mb loop to avoid head-of-line blocking
    # on the in-order engines. ----
    qk_proj(xk_t, wkt_sb, bk_sb, k_sb, 0, True)
    qk_proj(xq_t, wqt_sb, bq_sb, q_sb, 0, False)

    deferred = {}  # (c4, mb) -> list of callables
    deferred.setdefault((0, 1), []).append(lambda: v_proj(0, True))

    def run_deferred(pos):
        for fn in deferred.pop(pos, []):
            fn()

    def make_tail(c4, px, xcat):
        cs = slice(c4 * 512, (c4 + 1) * 512)
        state = {}

        def cp_step(h, use_act):
            def fn():
                cpdt = mybir.dt.float16 if c4 == 3 else f32
                cp = work.tile([65, 512], cpdt, tag="cp",
                               name=f"cp{c4}_{h}")
                if use_act:
                    nc.scalar.copy(cp, px[h][0:65, :])
                else:
                    nc.vector.tensor_copy(cp, px[h][0:65, :])
                state[h] = cp
            return fn

        def bcast_step(h):
            def fn():
                if c4 < 3:
                    # off-PE: bounce the sums row via DRAM, broadcast-read
                    s_dram = dpool.tile([1, 512], f32,
                                        tag="sd", name=f"sd{c4}_{h}")
                    nc.sync.dma_start(s_dram, state[h][64:65, :])
                    rbs = work.tile([64, 512], f32, tag="rbs",
                                    name=f"rbs{c4}_{h}")
                    srcb = bass.AP(tensor=s_dram.tensor, offset=s_dram.offset,
                                   ap=[[0, 64]] + list(s_dram.ap[1:]))
                    nc.sync.dma_start(rbs, srcb)
                    state[(h, "rb")] = rbs
                else:
                    rb = psPR.tile([128, 512], f32, tag="pr",
                                   name=f"rb{c4}_{h}")
                    nc.tensor.matmul(
                        rb[0:64, :], lhsT=ones_h[64:65, :],
                        rhs=state[h][64:65, :],
                        start=True, stop=True, tile_position=(64, 0))
                    state[(h, "rb")] = rb[0:64, :]
            return fn

        def norm_step(h):
            def fn():
                cp = state[h]
                rb = state[(h, "rb")]
                rcf = work.tile([64, 512], f32, tag="rcf", name=f"rcf{c4}_{h}")
                if rb.dtype != f32:
                    nc.vector.tensor_copy(rcf, rb)
                    rb = rcf
                rc = work.tile([64, 512], f32, tag="rc", name=f"rc{c4}_{h}")
                nc.vector.reciprocal_approx_fast(rc, rb)
                meng = nc.vector if (c4 == 3 and h == 1) else nc.gpsimd
                meng.tensor_tensor(
                    xcat[h * 64:(h + 1) * 64, :], cp[0:64, :], rc, op=MUL)
            return fn

        def oproj_step(oc, use_act):
            def fn():
                po = psPR.tile([128, 512], f32, tag="pr", name=f"po{c4}_{oc}")
                nc.tensor.matmul(
                    po, lhsT=wmcat_sb[:, 0, oc * 128:(oc + 1) * 128],
                    rhs=xcat, start=True, stop=True)
                ob = outp.tile([128, 512], f32, tag="ob", name=f"ob{c4}_{oc}")
                if use_act:
                    nc.scalar.copy(ob, po)
                else:
                    nc.vector.tensor_copy(ob, po)
                eng = nc.gpsimd if oc == 0 else nc.sync
                eng.dma_start(io["out"][oc * 128:(oc + 1) * 128, cs], ob)
            return fn

        return [cp_step(0, True), bcast_step(0), cp_step(1, False),
                bcast_step(1), norm_step(0), norm_step(1),
                oproj_step(0, True), oproj_step(1, False)]

    for w in range(1, NW):
        deferred.setdefault((0, 4 * w), []).extend([
            (lambda w=w: qk_proj(xk_t, wkt_sb, bk_sb, k_sb, w,
                                 use_act=(w % 2 == 0))),
            (lambda w=w: v_proj(w, use_act=(w % 2 == 1))),
        ])

    for c4 in range(4):
        cs = slice(c4 * 512, (c4 + 1) * 512)
        px = [psPX.tile([128, 512], f32, tag="px", name=f"px{c4}_{h}")
              for h in range(2)]
        xcat = xcp.tile([128, 512], bf16, tag="xc", name=f"xc{c4}")
        ptj = None
        pt_done = {}

        def emit_dr(j, _px=px, _pt=pt_done):
            for h in range(2):
                if CFG_DR:
                    nc.tensor.matmul(
                        _px[h][0:80, :],
                        lhsT=vt[:, j, h, :, :],
                        rhs=_pt[j][:, :, h, :],
                        start=(j == 0), stop=(j == 7),
                        perf_mode=DR,
                    )
                else:
                    for i in range(2):
                        nc.tensor.matmul(
                            _px[h][0:80, :],
                            lhsT=vt[:, j, h, i, :],
                            rhs=_pt[j][:, i, h, :],
                            start=(j == 0 and i == 0),
                            stop=(j == 7 and i == 1),
                        )

        for mb in range(MB):
            run_deferred((c4, mb))
            j = mb // 2
            if mb % 2 == 0:
                ptj = ptp.tile([128, 2, 2, 512], fp8, tag="pt",
                               name=f"pt{c4}_{j}")
            sc = psSC.tile([128, 1024], f32, tag="sc", name=f"sc{c4}_{mb}")
            if CFG_FILLER:
                nc.tensor.matmul(
                    sc[:, 0:512],
                    lhsT=k_sb[0:64, mb * 128:(mb + 1) * 128],
                    rhs=q_sb[0:64, cs],
                    start=True, stop=True, skip_group_check=True,
                    tile_position=(0, 0) if CFG_TILEPOS else None)
            for h in range(2):
                nc.tensor.matmul(
                    sc[:, h * 512:(h + 1) * 512],
                    lhsT=k_sb[h * 64:(h + 1) * 64, mb * 128:(mb + 1) * 128],
                    rhs=q_sb[h * 64:(h + 1) * 64, cs],
                    start=True, stop=True,
                    tile_position=(h * 64, 0) if CFG_TILEPOS else None,
                )
            use_act = (((j + c4) % 8) not in (1, 4, 6))
            if not CFG_SCHRAUD:
                use_act = True
            elif not CFG_ACT_FP8:
                use_act = False
            pslice = ptj[:, mb % 2, :, :]
            if use_act:
                nc.scalar.activation(pslice, sc, EXP, scale=0.125, bias=ln2b)
            else:
                nc.vector.tensor_scalar(
                    pslice.bitcast(i8), sc, SCHRAUD_A, SCHRAUD_B, MUL, ADD)
            if mb % 2 == 1:
                for h in range(2):
                    if CFG_DR:
                        nc.tensor.matmul(
                            px[h][0:80, :],
                            lhsT=vt[:, j, h, :, :],
                            rhs=ptj[:, :, h, :],
                            start=(j == 0), stop=(j == 7),
                            perf_mode=DR,
                        )
                    else:
                        for i in range(2):
                            nc.tensor.matmul(
                                px[h][0:80, :],
                                lhsT=vt[:, j, h, i, :],
                                rhs=ptj[:, i, h, :],
                                start=(j == 0 and i == 0),
                                stop=(j == 7 and i == 1),
                            )

        tail_ops = make_tail(c4, px, xcat)
        if c4 < 3:
            positions = [1, 2, 2, 3, 8, 9, 11, 13]
            for k, fn in enumerate(tail_ops):
                deferred.setdefault((c4 + 1, positions[k]), []).append(fn)
        else:
            for fn in tail_ops:
                fn()
        if c4 < 3:
            deferred.setdefault((c4 + 1, 0), []).append(
                lambda w=c4 + 1: qk_proj(xq_t, wqt_sb, bq_sb, q_sb, w,
                                         use_act=(w % 2 == 1)))

    if "dbg_q" in io:
        nc.sync.dma_start(io["dbg_q"], q_sb)
        nc.sync.dma_start(io["dbg_k"], k_sb)
        nc.sync.dma_start(io["dbg_vt"], vt.bitcast(i8))


def _build_nc(debug_dumps=False):
    key = ("nc", debug_dumps)
    if key in _CACHE:
        return _CACHE[key]
    from contextlib import ExitStack

    import concourse.mybir as mybir
    import concourse.tile as tile
    from concourse import bacc

    f32 = mybir.dt.float32
    bf16 = mybir.dt.bfloat16
    i8 = mybir.dt.int8
    nc = bacc.Bacc("TRN2", target_bir_lowering=False, debug=False, num_devices=8)
    io = {}
    for name, shape, dt_ in (
        ("xq", [128, 4, 2, 512], bf16),
        ("xk", [128, 4, 2, 512], bf16),
        ("xv", [128, 4, 2, 512], bf16),
        ("wqt", [128, 2, 128], bf16),
        ("wkt", [128, 2, 128], bf16),
        ("wvt", [128, 2, 128], bf16),
        ("bqk", [128, 2], f32),
        ("wmcat", [128, 2, 256], bf16),
    ):
        io[name] = nc.dram_tensor(name, shape, dt_, kind="ExternalInput").ap()
    io["out"] = nc.dram_tensor("out", [256, 2048], f32, kind="ExternalOutput").ap()
    if debug_dumps:
        io["dbg_q"] = nc.dram_tensor("dbg_q", [128, 2048], bf16, kind="ExternalOutput").ap()
        io["dbg_k"] = nc.dram_tensor("dbg_k", [128, 2048], bf16, kind="ExternalOutput").ap()
        io["dbg_vt"] = nc.dram_tensor("dbg_vt", [128, 8, 2, 2, 80], i8, kind="ExternalOutput").ap()

    with tile.TileContext(nc) as tc:
        with ExitStack() as ctx:
            _emit(ctx, tc, io)
    nc.compile()
    _CACHE[key] = nc
    _CACHE[(key, "io")] = io
    return nc


def make_in_maps(query, key, value, wq, bq, wk, bk, wv, bv, wm, bm):
    fb = lambda a: np.ascontiguousarray(np.asarray(a, dtype=np.float32).astype(BF))
    f = lambda a: np.ascontiguousarray(np.asarray(a), dtype=np.float32)
    query, key, value = f(query), f(key), f(value)
    wq, wk, wv, wm = f(wq), f(wk), f(wv), f(wm)
    bq, bk, bv = f(bq), f(bk), f(bv)

    def win(x):
        # [256, 2048] -> [128 p, 4 w, 2 cc, 512] with channel = cc*128 + p
        return fb(x.reshape(2, 128, 4, 512).transpose(1, 2, 0, 3))

    def wt(w, idx):
        # [256 in, 128 out(hd)] -> [128 p, 2 cc, 128 o]
        return fb(w[idx].T.reshape(2, 128, 128).transpose(1, 0, 2))

    def wmcat_host(wm, idx):
        # [128 p, 2, 256 o]; slot 0 rows = stacked (h, d) channels
        arr = np.zeros((128, 2, 256), dtype=np.float32)
        arr[:, 0, :] = wm[:, idx].T
        return fb(arr)

    in_maps = []
    for c in range(8):
        b, pair = divmod(c, 2)
        hs = (2 * pair, 2 * pair + 1)
        idx = np.array([d * H + h for h in hs for d in range(DIM)])
        m = {
            "xq": win(query[b]),
            "xk": win(key[b]),
            "xv": win(value[b]),
            "wqt": wt(wq, idx),
            "wkt": wt(wk, idx),
            "wvt": wt(wv, idx),
            "bqk": f(np.stack([bq[idx], bk[idx]], axis=1)),
            "wmcat": wmcat_host(wm, idx),
        }
        in_maps.append(m)
    return in_maps


def run(in_maps, trace=False, **kw):
    from concourse import bass_utils

    nc = _build_nc()
    return bass_utils.run_bass_kernel_spmd(
        nc, in_maps, core_ids=list(range(8)), trace=trace, **kw
    )


def gather(results, wm, bv, bm):
    # v-bias folded: out = wm @ (x/S) + (wm @ bv + bm)
    bm_eff = (np.asarray(bm, dtype=np.float32)
              + np.asarray(wm, dtype=np.float32) @ np.asarray(bv, dtype=np.float32))
    outs = [np.asarray(r["out"], dtype=np.float32) for r in results]
    return np.stack([outs[2 * b] + outs[2 * b + 1] + bm_eff[:, None]
                     for b in range(B)])


def kernel(query, key, value, wq, bq, wk, bk, wv, bv, wm, bm):
    in_maps = make_in_maps(query, key, value, wq, bq, wk, bk, wv, bv, wm, bm)
    res = run(in_maps)
    return gather(res.results, wm, bv, bm)
